# revision 21
# baseline (speedup 1.0000x reference)
"""Trainium2 Bass kernel for an additive-attention (GAT-style) head.

Reference math (N=8192, EMB=256, NHID=64, alpha=0.2):
    h      = input @ W                               [N, 64]
    s_src  = h @ a[:64];  s_dst = h @ a[64:]         [N]
    e      = leaky_relu(s_src[:,None] + s_dst[None,:], 0.2)
    att    = softmax(where(adj > 0, e, -9e15), axis=1)
    out    = att @ h                                 [N, 64]

Key algebraic restructuring (no transcendental ever touches the NxN matrix):
    exp(lrelu(t)) = max(exp(t), exp(alpha*t)) and both branches are rank-1 in
    (i, j).  Dividing row i by exp(s_src_i) (cancels in softmax):
        tau_ij = adj_ij * max(v_j, r_i * u_j)
    with r_i = exp((alpha-1)*s_src_i), v_j = exp(s_dst_j),
    u_j = v_j*w_j = exp(alpha*s_dst_j).
    Then out_i = (tau_i: @ h) / (tau_i: @ 1) with payload columns (h | 1) --
    the v factor lives entirely in the mask, computed by one fused DVE
    tensor_scalar with two per-partition operands.

Distribution: 1-D row partition of N across 8 cores (1024 rows each).  Each
core gets its adj shard TRANSPOSED and pre-cast to bf16 on the host
([8192, 1024]) so j lives on SBUF partitions and the att@h contraction runs
on TensorEngine without on-device transposes or dtype casts.  Instead of
computing h locally and AllGather-ing it (collective barrier + gather +
strided readback cost ~85us), every core redundantly computes the full
[N, 65] payload (v*h | v) from a replicated bf16 input^T (4MB): 64 j-tiles,
each two [128k x 128j] stationary matmuls against W_ext [128k, 65]
(W | W@a_dst), with the exp/v-scaling post-ops on the otherwise-idle Scalar
engine.

Per-core main loop over 64 j-tiles of [128, 1024]:
    sync DMA (bf16 adj tile)  ->  DVE tensor_scalar kap=max(r*w,1) (4x mode)
    ->  DVE tensor_tensor p=kap*adj (2x mode)  ->  PE matmul accumulate into
    [65, 1024] PSUM.
Postlude: PE transpose, softmax normalize, DMA out.
"""

import sys

sys.path.insert(0, "/opt/trn_rl_repo")

import ml_dtypes
import numpy as np
from contextlib import ExitStack

import concourse.bass as bass
import concourse.mybir as mybir
import concourse.tile as tile

N = 8192
EMB = 256
NHID = 64
ALPHA = 0.2
NCORES = 8
NLOC = N // NCORES          # 1024 rows per core
NT = N // 128               # 64 j-tiles
NHE = NHID + 1              # h plus v column (for the softmax denominator)
FP32 = mybir.dt.float32
BF16 = mybir.dt.bfloat16

AX = mybir.AxisListType
ALU = mybir.AluOpType
ACTF = mybir.ActivationFunctionType


class WaitSplitTileContext(tile.TileContext):
    """walrus' S3_LW (ldweights/matmul) struct accepts only ONE sync-wait
    command; Tile can emit matmuls with several.  Hoist the excess waits onto
    standalone InstEventSemaphore instructions on the same engine, inserted
    immediately before the matmul in the final scheduled order."""

    _NO_SPLIT_TYPES = (
        mybir.InstDrain,
        mybir.InstEventSemaphore,
    )

    def _add_instruction(self, inst):
        si = getattr(inst, "sync_info", None)
        if (
            si is not None
            and si.on_wait
            and len(si.on_wait) > 1
            and not isinstance(inst, self._NO_SPLIT_TYPES)
        ):
            waits = list(si.on_wait)
            for i, w in enumerate(waits[:-1]):
                ev = mybir.InstEventSemaphore(
                    name=f"{inst.name}-wsplit{i}",
                    engine=inst.engine,
                    ins=[],
                    outs=[],
                    sync_info=mybir.SyncInfo(on_wait=[w], on_update=[]),
                    bass_nofuse=True,
                )
                super()._add_instruction(ev)
            inst.sync_info = mybir.SyncInfo(
                on_wait=[waits[-1]], on_update=list(si.on_update)
            )
        super()._add_instruction(inst)

    def _drain_and_barrier(self, tick_clock, wait_clock):
        # The stock version attaches every engine's final tick as waits on ONE
        # drain -- over walrus' per-instruction limit.  Compute the waits on a
        # probe instruction, emit them as single-wait EventSemaphores on the
        # sync queue, then a clean drain.
        from concourse.vector_clock import ScopedClock

        probe = mybir.InstEventSemaphore(
            name=f"drain-wsplit-probe-{self.nc.next_id()}",
            engine=mybir.EngineType.SP,
            ins=[],
            outs=[],
            sync_info=None,
            bass_nofuse=True,
        )
        wait_clock.add_sem_waits(probe, ScopedClock({None: tick_clock.global_clock}))
        waits = list(probe.sync_info.on_wait) if probe.sync_info else []
        for i, w in enumerate(waits):
            ev = mybir.InstEventSemaphore(
                name=f"drain-wsplit{i}-{self.nc.next_id()}",
                engine=mybir.EngineType.SP,
                ins=[],
                outs=[],
                sync_info=mybir.SyncInfo(on_wait=[w], on_update=[]),
                bass_nofuse=True,
            )
            self._add_instruction(ev)
        self.nc.sync.drain()

        self.nc.all_engine_barrier()
        assert self.sems is not None
        popped = self.nc._tile_sem_poison_stack.pop()
        assert popped is self._sem_poison
        self.nc.clear_and_free_semaphores(list(self.sems.allocated().values()))
        self.nc.all_engine_barrier()


def build_kernel() -> bass.Bass:
    nc = bass.Bass(num_devices=NCORES)

    adjT = nc.declare_dram_parameter("adjT", [N, NLOC], BF16, isOutput=False)
    inTl_p = nc.declare_dram_parameter("inTl", [EMB, NLOC], BF16, isOutput=False)
    inTf_p = nc.declare_dram_parameter("inTf", [EMB, N], BF16, isOutput=False)
    W_p = nc.declare_dram_parameter("W", [EMB, NHID], FP32, isOutput=False)
    WT_p = nc.declare_dram_parameter("WT", [NHID, EMB], FP32, isOutput=False)
    a_p = nc.declare_dram_parameter("a", [2 * NHID], FP32, isOutput=False)
    ident_p = nc.declare_dram_parameter("ident", [128, 128], FP32, isOutput=False)
    # [p, ic, e] layout == out_sb SBUF layout; host untangles (pure transpose)
    out_p = nc.declare_dram_parameter("out", [128, 8 * NHID], FP32, isOutput=True)

    with WaitSplitTileContext(nc) as tc, ExitStack() as ctx:
        const = ctx.enter_context(tc.tile_pool(name="const", bufs=1))
        ps_scr = ctx.enter_context(
            tc.tile_pool(name="ps_scr", bufs=2, space=bass.MemorySpace.PSUM)
        )
        ps_h = ctx.enter_context(
            tc.tile_pool(name="ps_h", bufs=3, space=bass.MemorySpace.PSUM)
        )
        ps_acc = ctx.enter_context(
            tc.tile_pool(name="ps_acc", bufs=1, space=bass.MemorySpace.PSUM)
        )
        adj_pool = ctx.enter_context(tc.tile_pool(name="adj", bufs=8))
        kap_pool = ctx.enter_context(tc.tile_pool(name="kap", bufs=10))
        p_pool = ctx.enter_context(tc.tile_pool(name="p", bufs=4))

        # ---- constant / preamble tiles ----
        inTl = [const.tile([128, NLOC], BF16, tag=f"inTl{k}", name=f"inTl{k}") for k in range(2)]
        # full input^T, 8 column-chunks per k-half so payload tiles can start
        # as soon as their chunk lands
        inTf = [
            [
                const.tile([128, NLOC], BF16, tag=f"inTf{k}_{cc}", name=f"inTf{k}_{cc}")
                for cc in range(8)
            ]
            for k in range(2)
        ]
        w_sb = [const.tile([128, NHID], FP32, tag=f"w{k}", name=f"w{k}") for k in range(2)]
        wext = [const.tile([128, NHE], BF16, tag=f"wext{k}", name=f"wext{k}") for k in range(2)]
        wt_sb = const.tile([NHID, EMB], FP32)
        a_src = const.tile([NHID, 1], FP32, tag="asrc")
        a_dst = const.tile([NHID, 1], FP32, tag="adst")
        ident = const.tile([128, 128], FP32)
        ones1 = const.tile([1, 128], BF16)
        wa_src = const.tile([128, 2], BF16, tag="wasrc")
        r_row = const.tile([1, NLOC], BF16)
        r_bc = const.tile([128, NLOC], BF16)
        u_cols = const.tile([128, NT], FP32, tag="ucols")
        v_cols = const.tile([128, NT], FP32, tag="vcols")
        pay = const.tile([128, NT * NHE], BF16)
        houT = const.tile([NHE, NLOC], FP32)
        out_sb = const.tile([128, 8 * NHID], FP32)

        # order matters: the serial wa->s_src->r_bc chain consumes WT/a/inTl
        # first, so those dispatch first on the sync queue
        # the sync queue keeps ONLY the tiny wa inputs ahead of the adj
        # stream: consumers wait on the per-queue completion counter, so
        # anything else here would gate the whole startup chain
        nc.sync.dma_start(wt_sb[:], WT_p[:])
        nc.sync.dma_start(a_src[:], a_p[0:NHID])
        nc.sync.dma_start(a_dst[:], a_p[NHID : 2 * NHID])
        for k in range(2):
            nc.scalar.dma_start(inTl[k][:], inTl_p[128 * k : 128 * (k + 1), :])
        for k in range(2):
            nc.scalar.dma_start(w_sb[k][:], W_p[128 * k : 128 * (k + 1), :])
        nc.scalar.dma_start(ident[:], ident_p[:])
        nc.vector.memset(ones1[:], 1.0)
        # payload col 64 of every tile stays 1.0 (softmax denominator column);
        # cols 0:64 are fully overwritten by the ScalarE h-copies, so only the
        # strided ones-column needs initialising (tiny, keeps DVE queue free)
        nc.vector.memset(pay[:].rearrange("p (t e) -> p t e", e=NHE)[:, :, NHID], 1.0)

        # ---- wa = W @ a_half for src and dst halves ----
        # (whole serial chain emitted before the bulk DMAs: its sem waits then
        # reference only the small preamble transfers)
        for half, asb in enumerate([a_src, a_dst]):
            for ec in range(2):
                ps = ps_scr.tile([128, 1], FP32, tag="scr", name=f"ps_wa{half}{ec}")
                nc.tensor.matmul(
                    ps[:], wt_sb[:, 128 * ec : 128 * (ec + 1)], asb[:],
                    start=True, stop=True,
                )
                if half == 0:
                    nc.scalar.copy(wa_src[:, ec : ec + 1], ps[:])
                else:
                    # W_ext column 64 = W @ a_dst (bf16)
                    nc.scalar.copy(wext[ec][:, NHID : NHID + 1], ps[:])
        for ec in range(2):
            nc.scalar.copy(wext[ec][:, 0:NHID], w_sb[ec][:])

        # ---- s_src (local rows) row; r = exp((a-1)*s_src), broadcast ----
        for ih in range(2):
            ps = ps_scr.tile([1, 512], FP32, tag="scr", name=f"ps_ss{ih}")
            for kc in range(2):
                nc.tensor.matmul(
                    ps[:], wa_src[:, kc : kc + 1],
                    inTl[kc][:, 512 * ih : 512 * (ih + 1)],
                    start=(kc == 0), stop=(kc == 1),
                )
            nc.scalar.activation(
                r_row[:, 512 * ih : 512 * (ih + 1)], ps[:], ACTF.Exp,
                scale=ALPHA - 1.0,
            )
        for ih in range(2):
            ps = ps_scr.tile([128, 512], FP32, tag="scr", name=f"ps_rb{ih}")
            nc.tensor.matmul(
                ps[:], ones1[:], r_row[:, 512 * ih : 512 * (ih + 1)],
                start=True, stop=True,
            )
            nc.scalar.copy(r_bc[:, 512 * ih : 512 * (ih + 1)], ps[:])

        # inTf chunks on the (otherwise idle) gpsimd SWDGE queue, emitted
        # after the r chain so nothing upstream waits on this 4MB stream
        for cc in range(8):
            for k in range(2):
                nc.gpsimd.dma_start(
                    inTf[k][cc][:],
                    inTf_p[128 * k : 128 * (k + 1), NLOC * cc : NLOC * (cc + 1)],
                )

        # ---- replicated payload: (h | 1) for ALL 64 j-tiles ----
        pay3 = pay[:].rearrange("p (t e) -> p t e", e=NHE)
        for t in range(NT):
            cc, ic = t // 8, t % 8
            ps = ps_h.tile([128, NHE], FP32, tag="ph", name=f"ps_h{t}")
            for kc in range(2):
                nc.tensor.matmul(
                    ps[:],
                    inTf[kc][cc][:, 128 * ic : 128 * (ic + 1)],
                    wext[kc][:],
                    start=(kc == 0), stop=(kc == 1),
                )
            # u = v*w = exp(alpha*s_dst), v = exp(s_dst); payload h copy --
            # all on the otherwise-idle ScalarE
            nc.scalar.activation(
                u_cols[:, t : t + 1], ps[:, NHID : NHID + 1], ACTF.Exp,
                scale=ALPHA,
            )
            nc.scalar.activation(v_cols[:, t : t + 1], ps[:, NHID : NHID + 1], ACTF.Exp)
            nc.scalar.activation(pay3[:, t, 0:NHID], ps[:, 0:NHID], ACTF.Copy)

        # ---- main loop over 64 j-tiles ----
        ps_out = ps_acc.tile([NHE, NLOC], FP32, tag="ps_out")
        for t in range(NT):
            adj_bf = adj_pool.tile([128, NLOC], BF16)
            nc.sync.dma_start(adj_bf[:], adjT[128 * t : 128 * (t + 1), :])
            kap = kap_pool.tile([128, NLOC], BF16)
            nc.vector.tensor_scalar(
                kap[:], r_bc[:],
                u_cols[:, t : t + 1], v_cols[:, t : t + 1],
                ALU.mult, ALU.max,
            )
            p = p_pool.tile([128, NLOC], BF16)
            nc.vector.tensor_mul(p[:], kap[:], adj_bf[:])
            for ih in range(2):
                nc.tensor.matmul(
                    ps_out[:, 512 * ih : 512 * (ih + 1)],
                    pay3[:, t, 0:NHE],
                    p[:, 512 * ih : 512 * (ih + 1)],
                    start=(t == 0), stop=(t == NT - 1),
                )

        # ---- normalize + transpose + store ----
        nc.vector.tensor_copy(houT[:], ps_out[:])
        for ic in range(8):
            ps_t = ps_scr.tile([128, NHE], FP32, tag="scr", name=f"ps_t{ic}")
            nc.tensor.transpose(
                ps_t[:], houT[:, 128 * ic : 128 * (ic + 1)], ident[:NHE, :NHE]
            )
            zrec = kap_pool.tile([128, 1], FP32, tag="zrec", name=f"zrec{ic}")
            nc.vector.reciprocal(zrec[:], ps_t[:, NHID : NHID + 1])
            nc.vector.tensor_scalar(
                out_sb[:, NHID * ic : NHID * (ic + 1)], ps_t[:, 0:NHID],
                zrec[:], None, ALU.mult,
            )
        nc.sync.dma_start(out_p[:], out_sb[:])

    return nc


def shard_inputs(input, adj, W, a):
    """Host-side sharding/layout prep. Returns in_maps for the 8 cores."""
    input = np.asarray(input, dtype=np.float32)
    adj = np.asarray(adj, dtype=np.int32)
    W = np.ascontiguousarray(np.asarray(W, dtype=np.float32))
    a = np.ascontiguousarray(np.asarray(a, dtype=np.float32))
    inputT = np.ascontiguousarray(input.T.astype(ml_dtypes.bfloat16))
    adjT = np.ascontiguousarray(adj.T.astype(ml_dtypes.bfloat16))
    WT = np.ascontiguousarray(W.T)
    ident = np.eye(128, dtype=np.float32)
    in_maps = []
    for c in range(NCORES):
        rows = slice(c * NLOC, (c + 1) * NLOC)
        in_maps.append(
            {
                "adjT": np.ascontiguousarray(adjT[:, rows]),
                "inTl": np.ascontiguousarray(inputT[:, rows]),
                "inTf": inputT,
                "W": W,
                "WT": WT,
                "a": a,
                "ident": ident,
            }
        )
    return in_maps


_CACHE = {}


def kernel(input, adj, W, a, _trace=False, _return_result=False):
    from concourse.bass_utils import run_bass_kernel_spmd

    if "nc" not in _CACHE:
        _CACHE["nc"] = build_kernel()
    nc = _CACHE["nc"]
    in_maps = shard_inputs(input, adj, W, a)
    res = run_bass_kernel_spmd(
        nc, in_maps, core_ids=list(range(NCORES)), trace=_trace
    )
    out = np.concatenate(
        [
            res.results[c]["out"]
            .reshape(128, 8, NHID)
            .transpose(1, 0, 2)
            .reshape(NLOC, NHID)
            for c in range(NCORES)
        ],
        axis=0,
    )
    if _return_result:
        return out, res
    return out


if __name__ == "__main__":
    rng = np.random.default_rng(0)
    inp = rng.standard_normal((N, EMB), dtype=np.float32)
    adj = rng.integers(0, 2, size=(N, N), dtype=np.int32)
    W = (rng.standard_normal((EMB, NHID)) * 0.05).astype(np.float32)
    a = (rng.standard_normal(2 * NHID) * 0.05).astype(np.float32)
    out = kernel(inp, adj, W, a)
    print(out.shape, out.dtype)
